# revision 4
# baseline (speedup 1.0000x reference)
"""Trainium2 Bass kernel for nn_FilteringActLayer (StyleGAN3-style filtered
leaky-relu: bias + 2x zero-insert upsample FIR (separable) + leaky-relu/gain/
clamp + separable FIR 2x downsample).

Strategy (pure data parallel, 1 sample per core on 8 cores):
  Per sample [C=128, H=128, W=128], per channel c:
    MM1 (PE, data-stationary): out1[w,h'] = sum_h xb[h,w] * U1T[h,h']
        -- computes the H-axis up-conv AND the h<->w transpose in one matmul.
    MM2 (PE): a_m = U1[tile_m,:] @ out1  -> [128 w', 266 h'] per tile,
        w'-tiles {0:128, 128:256, 138:266} (overlapped 3rd tile keeps every
        matmul / evacuation full 128 partitions).
    ACT (ScalarE): PSUM evacuation fused with Lrelu(gain*z, alpha=slope),
        bf16 out.  (Optional DVE clamp pass when the analytic bound says the
        clamp can actually fire.)
    MM3 (PE): out3 = sum_k dnt_k.T @ a_k    (down-conv along w', K=266 split
        into 3 chunks with double-covered rows zeroed in the weights)
    T: per-channel 128x128 transposes of out3 chunks (DMA xbar or PE).
    MM4 (PE): y = sum_k dnt_k.T @ t_k       (down-conv along h')
  DRAM layouts are [h, c, w] in / [h'', c, w''] out; the host transposes
  to/from the reference [c, h, w] layout (host marshaling, not on device).
"""

import numpy as np
import ml_dtypes

UP = 2
PAD_LO, PAD_HI = 11, 10
TAPS = 12
N_CORES = 8
C, H, W = 128, 128, 128
P = 128
HP = 266  # upsampled axis length
G = 8     # channels per group
NG = C // G

# partition tiles of the 266 axis (3rd tile overlaps so all are 128 wide)
TILES = [(0, 128), (128, 256), (138, 266)]
# coverage for down-conv K chunks (zero the double-covered rows)
COVER = [(0, 128), (128, 138), (138, 266)]

T_MODE = "xbar"  # "xbar" (DMA transpose engine) or "pe" (TensorE transpose)

BF16 = ml_dtypes.bfloat16


def _build_u1(up_filter):
    fu2 = np.asarray(up_filter, np.float64) * UP
    o = np.arange(HP)[:, None]
    j = np.arange(H)[None, :]
    t = o - 2 * j
    U1 = np.where((t >= 0) & (t < TAPS), fu2[np.clip(t, 0, TAPS - 1)], 0.0)
    return U1.astype(np.float32)


def _build_dn(down_filter):
    fd = np.asarray(down_filter, np.float64)
    m = np.arange(H)[:, None]
    q = np.arange(HP)[None, :]
    t = q - 2 * m
    Dn = np.where((t >= 0) & (t < TAPS), fd[::-1][np.clip(t, 0, TAPS - 1)], 0.0)
    return Dn.astype(np.float32)


def _build_dnt_chunks(Dn):
    DnT = Dn.T  # [266, 128]
    out = np.zeros((P, 3, P), np.float32)  # [k-row, chunk, w'']
    for j, ((t0, t1), (c0, c1)) in enumerate(zip(TILES, COVER)):
        ch = DnT[t0:t1].copy()
        keep = np.zeros(t1 - t0, bool)
        keep[c0 - t0:c1 - t0] = True
        ch[~keep] = 0.0
        out[:, j, :] = ch
    return out


_CACHE = {}


def _build_bass(bias_vals, gain, slope, do_clamp, clamp):
    import concourse.bacc as bacc
    import concourse.mybir as mybir
    from concourse import tile

    f32 = mybir.dt.float32
    bf16 = mybir.dt.bfloat16
    AF = mybir.ActivationFunctionType
    ALU = mybir.AluOpType

    nc = bacc.Bacc(None, target_bir_lowering=False, debug=False)

    x_d = nc.dram_tensor("x", [P, C, W], f32, kind="ExternalInput")
    u1t_d = nc.dram_tensor("u1t", [P, HP], bf16, kind="ExternalInput")
    dnt_d = nc.dram_tensor("dnt", [P, 3, P], bf16, kind="ExternalInput")
    y_d = nc.dram_tensor("y", [P, C, W], f32, kind="ExternalOutput")
    if T_MODE == "pe":
        ident_d = nc.dram_tensor("ident", [P, P], bf16, kind="ExternalInput")

    with tile.TileContext(nc) as tc:
        with (
            tc.tile_pool(name="const", bufs=1) as const,
            tc.tile_pool(name="xb_p", bufs=2) as xb_p,
            tc.tile_pool(name="z1_p", bufs=2) as z1_p,
            tc.tile_pool(name="a_p", bufs=2) as a_p,
            tc.tile_pool(name="m3_p", bufs=2) as m3_p,
            tc.tile_pool(name="tt_p", bufs=2) as tt_p,
            tc.tile_pool(name="yo_p", bufs=2) as yo_p,
            tc.tile_pool(name="ps_b", bufs=2, space="PSUM") as ps_b,
            tc.tile_pool(name="ps_s", bufs=2, space="PSUM") as ps_s,
        ):
            u1t = const.tile([P, HP], bf16)
            nc.sync.dma_start(u1t[:], u1t_d[:])
            dnt = const.tile([P, 3, P], bf16)
            nc.sync.dma_start(dnt[:], dnt_d[:])
            if T_MODE == "pe":
                ident = const.tile([P, P], bf16)
                nc.sync.dma_start(ident[:], ident_d[:])

            for g in range(NG):
                cg = g * G
                # ---- load + cast via SWDGE, then per-channel bias add ----
                xb = xb_p.tile([P, G, W], bf16)
                nc.gpsimd.dma_start(xb[:], x_d[:, cg:cg + G, :])
                for ci in range(G):
                    nc.vector.tensor_scalar(
                        out=xb[:, ci, :], in0=xb[:, ci, :],
                        scalar1=float(bias_vals[cg + ci]), scalar2=None,
                        op0=ALU.add)

                # ---- MM1: per channel, out1[w, h'] in psum; 3-ch batches ----
                z1 = z1_p.tile([P, G, HP], bf16)
                for c0 in range(0, G, 3):
                    nb = min(3, G - c0)
                    ps1 = ps_b.tile([P, 3, 512], f32, tag="ps_b")
                    for i in range(nb):
                        nc.tensor.matmul(
                            ps1[:, i, :HP], lhsT=xb[:, c0 + i, :], rhs=u1t[:],
                            start=True, stop=True)
                    nc.vector.tensor_copy(
                        out=z1[:, c0:c0 + nb, :], in_=ps1[:, :nb, :HP])

                # ---- MM2 + fused Lrelu evacuation (3-ch batches) ----
                z1f = z1[:].rearrange("p g h -> p (g h)")
                a = a_p.tile([P, 3, G * HP], bf16)
                for m in range(3):
                    t0, t1 = TILES[m]
                    for c0 in range(0, G, 3):
                        nb = min(3, G - c0)
                        ps2 = ps_b.tile([P, 3, 512], f32, tag="ps_b")
                        for i in range(nb):
                            n0 = (c0 + i) * HP
                            nc.tensor.matmul(
                                ps2[:, i, :HP], lhsT=u1t[:, t0:t1],
                                rhs=z1f[:, n0:n0 + HP], start=True, stop=True)
                        dst = a[:, m, c0 * HP:(c0 + nb) * HP].rearrange(
                            "p (b h) -> p b h", h=HP)
                        nc.scalar.activation(
                            out=dst, in_=ps2[:, :nb, :HP], func=AF.Prelu,
                            bias=0.0, scale=float(gain), alpha=float(slope))
                if do_clamp:
                    for m in range(3):
                        nc.vector.tensor_scalar(
                            out=a[:, m, :], in0=a[:, m, :],
                            scalar1=float(clamp), scalar2=float(-clamp),
                            op0=ALU.min, op1=ALU.max)

                # ---- MM3: down-conv along w' (K=266 via 3 masked chunks) ----
                m3 = m3_p.tile([P, G, HP], bf16)
                for c0 in range(0, G, 3):
                    nb = min(3, G - c0)
                    ps3 = ps_b.tile([P, 3, 512], f32, tag="ps_b")
                    for i in range(nb):
                        n0 = (c0 + i) * HP
                        for k in range(3):
                            nc.tensor.matmul(
                                ps3[:, i, :HP], lhsT=dnt[:, k, :],
                                rhs=a[:, k, n0:n0 + HP],
                                start=(k == 0), stop=(k == 2))
                    nc.vector.tensor_copy(
                        out=m3[:, c0:c0 + nb, :], in_=ps3[:, :nb, :HP])

                # ---- T: per-channel 128x128 transposes of out3 chunks ----
                tts = []
                for k in range(3):
                    tt_tile = tt_p.tile([P, G, W], bf16, tag=f"tt{k}")
                    tts.append(tt_tile)
                if T_MODE == "xbar":
                    for k in range(3):
                        t0, t1 = TILES[k]
                        for ci in range(G):
                            nc.sync.dma_start(
                                out=tts[k][:, ci, :], in_=m3[:, ci, t0:t1],
                                transpose=True)
                else:
                    for k in range(3):
                        t0, t1 = TILES[k]
                        for c0 in range(0, G, 4):
                            pst = ps_s.tile([P, 4, P], f32, tag="ps_s")
                            for i in range(4):
                                nc.tensor.transpose(
                                    pst[:, i, :], m3[:, c0 + i, t0:t1], ident[:])
                            nc.vector.tensor_copy(
                                out=tts[k][:, c0:c0 + 4, :], in_=pst[:])

                # ---- MM4: down-conv along h' + store ----
                yo = yo_p.tile([P, G * W], f32)
                ttf = [t[:].rearrange("p g w -> p (g w)") for t in tts]
                for n0 in range(0, G * W, 512):
                    ps4 = ps_s.tile([P, 4, P], f32, tag="ps_s")
                    ps4f = ps4[:].rearrange("p a b -> p (a b)")
                    for k in range(3):
                        nc.tensor.matmul(
                            ps4f[:, :], lhsT=dnt[:, k, :],
                            rhs=ttf[k][:, n0:n0 + 512],
                            start=(k == 0), stop=(k == 2))
                    nc.scalar.copy(out=yo[:, n0:n0 + 512], in_=ps4f[:, :])
                nc.sync.dma_start(
                    out=y_d[:, cg:cg + G, :],
                    in_=yo[:].rearrange("p (g w) -> p g w", w=W))

    nc.compile()
    return nc


def kernel(x, b, up_filter, down_filter, gain, slope, clamp):
    from concourse.bass_utils import run_bass_kernel_spmd

    x = np.asarray(x, np.float32)
    b = np.asarray(b, np.float32)
    up_filter = np.asarray(up_filter, np.float32)
    down_filter = np.asarray(down_filter, np.float32)
    gain = float(np.asarray(gain)); slope = float(np.asarray(slope))
    clamp = float(np.asarray(clamp))
    assert gain > 0.0, "kernel assumes gain > 0 (Lrelu scale folding)"

    U1 = _build_u1(up_filter)
    dnt = _build_dnt_chunks(_build_dn(down_filter))

    # can the clamp ever fire?  conservative L1 bound on pre-clamp values
    l1 = float(np.abs(up_filter * UP).sum())
    xmax = float(np.abs(x + b[None, :, None, None]).max())
    do_clamp = bool(xmax * l1 * l1 * abs(gain) >= 0.98 * clamp)

    key = (tuple(np.round(b, 7)), round(gain, 9), round(slope, 9),
           do_clamp, round(clamp, 6), T_MODE)
    if key not in _CACHE:
        _CACHE[key] = _build_bass(b, gain, slope, do_clamp, clamp)
    nc = _CACHE[key]

    u1t_np = np.ascontiguousarray(U1.T).astype(BF16)          # [128, 266]
    dnt_np = dnt.astype(BF16)                                  # [128, 3, 128]
    in_maps = []
    for n in range(N_CORES):
        m = {"x": np.ascontiguousarray(x[n].transpose(1, 0, 2)),
             "u1t": u1t_np, "dnt": dnt_np}
        if T_MODE == "pe":
            m["ident"] = np.eye(P, dtype=np.float32).astype(BF16)
        in_maps.append(m)

    res = run_bass_kernel_spmd(nc, in_maps, core_ids=list(range(N_CORES)))
    global LAST_RESULT
    LAST_RESULT = res
    out = np.stack([r["y"].transpose(1, 0, 2) for r in res.results])
    return out.astype(np.float32)


LAST_RESULT = None


if __name__ == "__main__":
    rng = np.random.default_rng(0)
    x = rng.standard_normal((N_CORES, C, H, W), np.float32)
    b = (rng.standard_normal(C) * 0.1).astype(np.float32)
    fu = rng.standard_normal(TAPS).astype(np.float32)
    fu /= np.abs(fu).sum()
    fd = rng.standard_normal(TAPS).astype(np.float32)
    fd /= np.abs(fd).sum()
    y = kernel(x, b, fu, fd, np.float32(np.sqrt(2)), np.float32(0.2),
               np.float32(256.0))
    print("kernel ran, output shape", y.shape)
